# revision 25
# baseline (speedup 1.0000x reference)
"""Behavior-specific FFN (MoE routing) Trainium2 kernel.

Strategy: expert-parallel with host-side routing. Tokens are gathered by
behavior id on the host (numpy), each behavior's tokens are split across
2 of the 8 NeuronCores, and every core runs a dense 2-layer FFN
(relu(x @ W1 + B1) @ W2 + B2) for its single behavior over its token
shard. The host scatters results back; padding tokens (behavior 0) stay
zero.

Device layout: tokens live on the matmul free (moving) dim, feature dims
on partitions. Layer 1: out[F_tile, tok] += W1[H_tile, F_tile].T @
xT[H_tile, tok]; layer 2 contracts over F the same way. x is fed
pre-transposed ([H, N]) by the host so no on-device transpose is needed.
"""

import numpy as np

_B, _T, _H, _F = 32, 512, 512, 2048
_NB = 4
_P = 128
_NCORES = 8
_TOK_TILE = 512

# Stash of the most recent BassKernelResults (exec_time_ns etc.) for the
# local test harness; harmless in the grading path.
LAST_RESULTS = None

_NC_CACHE = {}


def _token_tiles(n_pad):
    """Chunk n_pad into token tiles: full 512s plus one 128-aligned remainder."""
    tiles = []
    off = 0
    while n_pad - off >= _TOK_TILE:
        tiles.append((off, _TOK_TILE))
        off += _TOK_TILE
    if n_pad - off:
        tiles.append((off, n_pad - off))
    return tiles


def _dedupe_ldweights(nc):
    """Remove Ldweights that re-load the exact weights already resident in
    the PE array (same AP, no intervening clobber, no sync conditions).
    The paired Matmults (ldweights=False) then use the already-loaded
    weights — this is the documented explicit-LDW + non-self-loading-MM
    hardware pattern (valid for bf16; NOT for fp32/fp32r)."""
    removed = 0
    for f in nc.m.functions:
        for blk in f.blocks:
            keep = []
            last_key = None
            for inst in blk.instructions:
                op = inst.opcode
                if op == "Ldweights":
                    x = inst.ins[0]
                    key = (
                        getattr(x, "memref", None),
                        str(getattr(x, "ap", None)),
                        getattr(x, "offset", None),
                    )
                    clean = not (inst.has_wait() or inst.has_update())
                    if clean and key == last_key:
                        removed += 1
                        continue
                    last_key = key
                elif op in ("Matmult", "EventSemaphore", "Nop", "Activation",
                            "TensorCopy", "TensorTensor", "TensorScalarPtr",
                            "DMACopy", "TensorReduce", "Memset"):
                    pass  # doesn't clobber the PE weight array
                else:
                    last_key = None
                keep.append(inst)
            if removed:
                blk.instructions[:] = keep
    return removed


def _build(n_pad, mm_dtype_name, repeats=1, loop=1, style="stream"):
    import os as _os
    _dev = _os.environ.get("BSPFF_DEV") == "1"
    sub_n = int(_os.environ.get("SUB_N", "512")) if _dev else 512
    skip_act = _dev and _os.environ.get("SKIP_ACT") == "1"
    skip_ydma = _dev and _os.environ.get("SKIP_YDMA") == "1"
    skip_xdma = _dev and _os.environ.get("SKIP_XDMA") == "1"
    evac = _os.environ.get("EVAC", "act") if _dev else "act"
    w_in_loop = _dev and _os.environ.get("W_IN_LOOP") == "1"

    from contextlib import ExitStack, nullcontext

    import concourse.bass as bass
    import concourse.mybir as mybir
    import concourse.tile as tile
    from concourse import bacc

    f32 = mybir.dt.float32
    mm_dt = getattr(mybir.dt, mm_dtype_name)
    AF = mybir.ActivationFunctionType
    KH = _H // _P   # 4  K-subtiles for layer 1 / M-tiles for layer 2
    MF = _F // _P   # 16 M-tiles for layer 1 / K-subtiles for layer 2

    nc = bacc.Bacc("TRN2", target_bir_lowering=False, debug=False, num_devices=_NCORES)
    xT = nc.dram_tensor("xT", [_H, n_pad], mm_dt, kind="ExternalInput").ap()
    w1 = nc.dram_tensor("w1", [_H, _F], mm_dt, kind="ExternalInput").ap()
    w2 = nc.dram_tensor("w2", [_F, _H], mm_dt, kind="ExternalInput").ap()
    b1 = nc.dram_tensor("b1", [_P, MF], f32, kind="ExternalInput").ap()
    b2 = nc.dram_tensor("b2", [_P, KH], f32, kind="ExternalInput").ap()
    yT = nc.dram_tensor("yT", [_H, n_pad], f32, kind="ExternalOutput").ap()

    with tile.TileContext(nc) as tc, ExitStack() as ctx:
        grouped = style == "grouped"
        consts = ctx.enter_context(tc.tile_pool(name="consts", bufs=1))
        xp = ctx.enter_context(tc.tile_pool(name="xp", bufs=2 if grouped else 3))
        hp = ctx.enter_context(tc.tile_pool(name="hp", bufs=1 if grouped else 2))
        yp = ctx.enter_context(tc.tile_pool(name="yp", bufs=2 if grouped else 3))
        pp = ctx.enter_context(tc.tile_pool(name="pp", bufs=8 if grouped else 4, space="PSUM"))

        w1s = consts.tile([_P, KH, _F], mm_dt)
        w2s = consts.tile([_P, MF, _H], mm_dt)
        w1r = w1.rearrange("(ko p) f -> p ko f", p=_P)
        w2r = w2.rearrange("(ko p) h -> p ko h", p=_P)

        def load_weights():
            # Chunk weight loads by output-column slice: the m-th matmul group
            # only needs its own 128-wide slice, so compute starts ~1-2 us in.
            for m in range(MF):
                nc.sync.dma_start(w1s[:, :, m * _P:(m + 1) * _P], w1r[:, :, m * _P:(m + 1) * _P])
            for m2 in range(KH):
                nc.sync.dma_start(w2s[:, :, m2 * _P:(m2 + 1) * _P], w2r[:, :, m2 * _P:(m2 + 1) * _P])

        if not w_in_loop:
            load_weights()
        b1s = consts.tile([_P, MF], f32)
        nc.sync.dma_start(b1s[:], b1)
        b2s = consts.tile([_P, KH], f32)
        nc.sync.dma_start(b2s[:], b2)

        xTr = xT.rearrange("(ko p) n -> p ko n", p=_P)
        yTr = yT.rearrange("(mo p) n -> p mo n", p=_P)

        assert n_pad % _P == 0
        tiles = _token_tiles(n_pad)
        loop_cm = (
            tc.For_i(0, loop, 1, hint_engines=(mybir.EngineType.PE, mybir.EngineType.Activation, mybir.EngineType.SP))
            if loop > 1
            else nullcontext()
        )
        with loop_cm:
          for _rep in range(repeats):
            if w_in_loop:
                load_weights()
            if style == "grouped":
                # Tokens-inner order: one weight tile feeds all token tiles
                # back-to-back, so redundant Ldweights can be dropped.
                xt = xp.tile([_P, KH, n_pad], mm_dt, tag="xt")
                for k in range(KH):
                    nc.sync.dma_start(xt[:, k, :], xTr[:, k, :])
                ht = hp.tile([_P, MF, n_pad], mm_dt, tag="ht")
                for m in range(MF):
                    pss = [
                        pp.tile([_P, tn], f32, tag="ps", name=f"ps_{m}_{i}")
                        for i, (t0, tn) in enumerate(tiles)
                    ]
                    for k in range(KH):
                        for i, (t0, tn) in enumerate(tiles):
                            nc.tensor.matmul(
                                pss[i][:],
                                w1s[:, k, m * _P:(m + 1) * _P],
                                xt[:, k, t0:t0 + tn],
                                start=(k == 0),
                                stop=(k == KH - 1),
                            )
                    for i, (t0, tn) in enumerate(tiles):
                        nc.scalar.activation(
                            ht[:, m, t0:t0 + tn], pss[i][:], AF.Relu, bias=b1s[:, m:m + 1]
                        )
                yt = yp.tile([_P, KH, n_pad], f32, tag="yt")
                for m2 in range(KH):
                    pss2 = [
                        pp.tile([_P, tn], f32, tag="ps", name=f"ps2_{m2}_{i}")
                        for i, (t0, tn) in enumerate(tiles)
                    ]
                    for k2 in range(MF):
                        for i, (t0, tn) in enumerate(tiles):
                            nc.tensor.matmul(
                                pss2[i][:],
                                w2s[:, k2, m2 * _P:(m2 + 1) * _P],
                                ht[:, k2, t0:t0 + tn],
                                start=(k2 == 0),
                                stop=(k2 == MF - 1),
                            )
                    for i, (t0, tn) in enumerate(tiles):
                        nc.scalar.activation(
                            yt[:, m2, t0:t0 + tn], pss2[i][:], AF.Identity, bias=b2s[:, m2:m2 + 1]
                        )
                for m2 in range(KH):
                    nc.sync.dma_start(yTr[:, m2, :], yt[:, m2, :])
                continue

            for t0, tn in tiles:
                sl = slice(t0, t0 + tn)
                nq = (tn + sub_n - 1) // sub_n
                xt = xp.tile([_P, KH, tn], mm_dt, tag="xt")
                if not skip_xdma:
                    nc.sync.dma_start(xt[:], xTr[:, :, sl])

                ht = hp.tile([_P, MF, tn], mm_dt, tag="ht")
                for m in range(MF):
                    ps = pp.tile([_P, tn], f32, tag="ps1")
                    for k in range(KH):
                        for q in range(nq):
                            qs = slice(q * sub_n, min((q + 1) * sub_n, tn))
                            nc.tensor.matmul(
                                ps[:, qs],
                                w1s[:, k, m * _P:(m + 1) * _P],
                                xt[:, k, qs],
                                start=(k == 0 and q == 0),
                                stop=(k == KH - 1 and q == nq - 1),
                                skip_group_check=True,
                            )
                    if not skip_act:
                        use_dve = evac == "dve" or (evac == "split" and m % 2 == 1)
                        if use_dve:
                            nc.vector.tensor_scalar(
                                ht[:, m, :], ps[:], b1s[:, m:m + 1], 0.0,
                                mybir.AluOpType.add, mybir.AluOpType.max,
                            )
                        else:
                            nc.scalar.activation(ht[:, m, :], ps[:], AF.Relu, bias=b1s[:, m:m + 1])

                yt = yp.tile([_P, KH, tn], f32, tag="yt")
                for m2 in range(KH):
                    ps2 = pp.tile([_P, tn], f32, tag="ps2")
                    for k2 in range(MF):
                        for q in range(nq):
                            qs = slice(q * sub_n, min((q + 1) * sub_n, tn))
                            nc.tensor.matmul(
                                ps2[:, qs],
                                w2s[:, k2, m2 * _P:(m2 + 1) * _P],
                                ht[:, k2, qs],
                                start=(k2 == 0 and q == 0),
                                stop=(k2 == MF - 1 and q == nq - 1),
                                skip_group_check=True,
                            )
                    if not skip_act:
                        use_dve = evac == "dve" or (evac == "split" and m2 % 2 == 1)
                        if use_dve:
                            nc.vector.tensor_scalar_add(yt[:, m2, :], ps2[:], b2s[:, m2:m2 + 1])
                        else:
                            nc.scalar.activation(yt[:, m2, :], ps2[:], AF.Identity, bias=b2s[:, m2:m2 + 1])
                if not skip_ydma:
                    nc.sync.dma_start(yTr[:, :, sl], yt[:])

    nc.compile()
    if style == "grouped":
        n_removed = _dedupe_ldweights(nc)
        import os as _os
        if _os.environ.get("DEBUG_LDW"):
            print(f"[kernel] deduped {n_removed} Ldweights")
    return nc


_MM_DTYPE = "float32r"
_MM_STYLE = "stream"


def _get_nc(n_pad, mm_dtype_name, repeats=1, loop=1, style="stream"):
    key = (n_pad, mm_dtype_name, repeats, loop, style)
    if key not in _NC_CACHE:
        _NC_CACHE[key] = _build(n_pad, mm_dtype_name, repeats, loop, style)
    return _NC_CACHE[key]


def _np_mm_dtype(mm_dtype_name):
    if mm_dtype_name == "bfloat16":
        import ml_dtypes

        return ml_dtypes.bfloat16
    return np.float32


def _prepare(x, b_seq, W1, B1, W2, B2, mm_dtype_name):
    """Host-side routing: returns (idx_per_core, n_pad, in_maps)."""
    np_dt = _np_mm_dtype(mm_dtype_name)
    x = np.asarray(x)
    flat_x = np.ascontiguousarray(x.reshape(-1, _H), dtype=np.float32)
    bs = np.asarray(b_seq).reshape(-1)

    # Route: behavior b -> cores 2b and 2b+1, tokens split evenly.
    idx_per_core = []
    for b in range(_NB):
        idx = np.nonzero(bs == b + 1)[0]
        h = (len(idx) + 1) // 2
        idx_per_core.append(idx[:h])
        idx_per_core.append(idx[h:])
    nmax = max(len(i) for i in idx_per_core)
    n_pad = max(_P, ((nmax + _P - 1) // _P) * _P)

    in_maps = []
    for c in range(_NCORES):
        beh = c // 2
        idx = idx_per_core[c]
        xT = np.zeros((_H, n_pad), np_dt)
        if len(idx):
            xT[:, :len(idx)] = flat_x[idx].T.astype(np_dt)
        in_maps.append({
            "xT": xT,
            "w1": np.ascontiguousarray(np.asarray(W1[beh]).astype(np_dt)),
            "w2": np.ascontiguousarray(np.asarray(W2[beh]).astype(np_dt)),
            "b1": np.ascontiguousarray(np.asarray(B1[beh], dtype=np.float32).reshape(_F // _P, _P).T),
            "b2": np.ascontiguousarray(np.asarray(B2[beh], dtype=np.float32).reshape(_H // _P, _P).T),
        })
    return idx_per_core, n_pad, in_maps


def kernel(x, b_seq, W1, B1, W2, B2, _repeats=1):
    global LAST_RESULTS
    import os

    from concourse.bass_utils import run_bass_kernel_spmd

    _dev = os.environ.get("BSPFF_DEV") == "1"
    mm_dtype = os.environ.get("MM_DTYPE", _MM_DTYPE) if _dev else _MM_DTYPE
    style = os.environ.get("MM_STYLE", _MM_STYLE) if _dev else _MM_STYLE
    idx_per_core, n_pad, in_maps = _prepare(x, b_seq, W1, B1, W2, B2, mm_dtype)
    nc = _get_nc(n_pad, mm_dtype, _repeats, style=style)

    res = run_bass_kernel_spmd(nc, in_maps, core_ids=list(range(_NCORES)))
    LAST_RESULTS = res

    out = np.zeros((_B * _T, _H), np.float32)
    for c in range(_NCORES):
        idx = idx_per_core[c]
        if len(idx):
            out[idx] = res.results[c]["yT"][:, :len(idx)].T
    return out.reshape(_B, _T, _H)


# revision 27
# speedup vs baseline: 1.1765x; 1.1765x over previous
"""Behavior-specific FFN (MoE routing) Trainium2 kernel.

Strategy: expert-parallel with host-side routing. Tokens are gathered by
behavior id on the host (numpy), each behavior's tokens are split across
2 of the 8 NeuronCores, and every core runs a dense 2-layer FFN
(relu(x @ W1 + B1) @ W2 + B2) for its single behavior over its token
shard. The host scatters results back; padding tokens (behavior 0) stay
zero.

Device layout: tokens live on the matmul free (moving) dim, feature dims
on partitions. Layer 1: out[F_tile, tok] += W1[H_tile, F_tile].T @
xT[H_tile, tok]; layer 2 contracts over F the same way. x is fed
pre-transposed ([H, N]) by the host so no on-device transpose is needed.
"""

import numpy as np

_B, _T, _H, _F = 32, 512, 512, 2048
_NB = 4
_P = 128
_NCORES = 8
_TOK_TILE = 512

# Stash of the most recent BassKernelResults (exec_time_ns etc.) for the
# local test harness; harmless in the grading path.
LAST_RESULTS = None

_NC_CACHE = {}


def _token_tiles(n_pad):
    """Chunk n_pad into token tiles, every tile in [256, 512] columns.

    fp32r matmuls only run at full rate with a moving dim >= 256, so the
    tail is split into two roughly-equal tiles instead of leaving a
    narrow remainder. n_pad itself is exact (no alignment padding)."""
    assert n_pad >= 256
    tiles = []
    off = 0
    rem = n_pad
    while rem > 1024:
        tiles.append((off, _TOK_TILE))
        off += _TOK_TILE
        rem -= _TOK_TILE
    if rem > 512:
        a = ((rem // 2 + 15) // 16) * 16
        tiles.append((off, a))
        tiles.append((off + a, rem - a))
    else:
        tiles.append((off, rem))
    return tiles


def _dedupe_ldweights(nc):
    """Remove Ldweights that re-load the exact weights already resident in
    the PE array (same AP, no intervening clobber, no sync conditions).
    The paired Matmults (ldweights=False) then use the already-loaded
    weights — this is the documented explicit-LDW + non-self-loading-MM
    hardware pattern (valid for bf16; NOT for fp32/fp32r)."""
    removed = 0
    for f in nc.m.functions:
        for blk in f.blocks:
            keep = []
            last_key = None
            for inst in blk.instructions:
                op = inst.opcode
                if op == "Ldweights":
                    x = inst.ins[0]
                    key = (
                        getattr(x, "memref", None),
                        str(getattr(x, "ap", None)),
                        getattr(x, "offset", None),
                    )
                    clean = not (inst.has_wait() or inst.has_update())
                    if clean and key == last_key:
                        removed += 1
                        continue
                    last_key = key
                elif op in ("Matmult", "EventSemaphore", "Nop", "Activation",
                            "TensorCopy", "TensorTensor", "TensorScalarPtr",
                            "DMACopy", "TensorReduce", "Memset"):
                    pass  # doesn't clobber the PE weight array
                else:
                    last_key = None
                keep.append(inst)
            if removed:
                blk.instructions[:] = keep
    return removed


def _build(n_pad, mm_dtype_name, repeats=1, loop=1, style="stream"):
    import os as _os
    _dev = _os.environ.get("BSPFF_DEV") == "1"
    sub_n = int(_os.environ.get("SUB_N", "512")) if _dev else 512
    skip_act = _dev and _os.environ.get("SKIP_ACT") == "1"
    skip_ydma = _dev and _os.environ.get("SKIP_YDMA") == "1"
    skip_xdma = _dev and _os.environ.get("SKIP_XDMA") == "1"
    evac = _os.environ.get("EVAC", "act") if _dev else "act"
    w_in_loop = _dev and _os.environ.get("W_IN_LOOP") == "1"

    from contextlib import ExitStack, nullcontext

    import concourse.bass as bass
    import concourse.mybir as mybir
    import concourse.tile as tile
    from concourse import bacc

    f32 = mybir.dt.float32
    mm_dt = getattr(mybir.dt, mm_dtype_name)
    AF = mybir.ActivationFunctionType
    KH = _H // _P   # 4  K-subtiles for layer 1 / M-tiles for layer 2
    MF = _F // _P   # 16 M-tiles for layer 1 / K-subtiles for layer 2

    nc = bacc.Bacc("TRN2", target_bir_lowering=False, debug=False, num_devices=_NCORES)
    xT = nc.dram_tensor("xT", [_H, n_pad], mm_dt, kind="ExternalInput").ap()
    w1 = nc.dram_tensor("w1", [_H, _F], mm_dt, kind="ExternalInput").ap()
    w2 = nc.dram_tensor("w2", [_F, _H], mm_dt, kind="ExternalInput").ap()
    b1 = nc.dram_tensor("b1", [_P, MF], f32, kind="ExternalInput").ap()
    b2 = nc.dram_tensor("b2", [_P, KH], f32, kind="ExternalInput").ap()
    yT = nc.dram_tensor("yT", [_H, n_pad], f32, kind="ExternalOutput").ap()

    with tile.TileContext(nc) as tc, ExitStack() as ctx:
        grouped = style == "grouped"
        consts = ctx.enter_context(tc.tile_pool(name="consts", bufs=1))
        xp = ctx.enter_context(tc.tile_pool(name="xp", bufs=2 if grouped else 3))
        hp = ctx.enter_context(tc.tile_pool(name="hp", bufs=1 if grouped else 2))
        yp = ctx.enter_context(tc.tile_pool(name="yp", bufs=2 if grouped else 3))
        pp = ctx.enter_context(tc.tile_pool(name="pp", bufs=8 if grouped else 4, space="PSUM"))

        w1s = consts.tile([_P, KH, _F], mm_dt)
        w2s = consts.tile([_P, MF, _H], mm_dt)
        w1r = w1.rearrange("(ko p) f -> p ko f", p=_P)
        w2r = w2.rearrange("(ko p) h -> p ko h", p=_P)

        def load_weights():
            # Chunk weight loads by output-column slice: the m-th matmul group
            # only needs its own 128-wide slice, so compute starts ~1-2 us in.
            for m in range(MF):
                nc.sync.dma_start(w1s[:, :, m * _P:(m + 1) * _P], w1r[:, :, m * _P:(m + 1) * _P])
            for m2 in range(KH):
                nc.sync.dma_start(w2s[:, :, m2 * _P:(m2 + 1) * _P], w2r[:, :, m2 * _P:(m2 + 1) * _P])

        if not w_in_loop:
            load_weights()
        b1s = consts.tile([_P, MF], f32)
        nc.sync.dma_start(b1s[:], b1)
        b2s = consts.tile([_P, KH], f32)
        nc.sync.dma_start(b2s[:], b2)

        xTr = xT.rearrange("(ko p) n -> p ko n", p=_P)
        yTr = yT.rearrange("(mo p) n -> p mo n", p=_P)

        tiles = _token_tiles(n_pad)
        loop_cm = (
            tc.For_i(0, loop, 1, hint_engines=(mybir.EngineType.PE, mybir.EngineType.Activation, mybir.EngineType.SP))
            if loop > 1
            else nullcontext()
        )
        with loop_cm:
          for _rep in range(repeats):
            if w_in_loop:
                load_weights()
            if style == "grouped":
                # Tokens-inner order: one weight tile feeds all token tiles
                # back-to-back, so redundant Ldweights can be dropped.
                xt = xp.tile([_P, KH, n_pad], mm_dt, tag="xt")
                for k in range(KH):
                    nc.sync.dma_start(xt[:, k, :], xTr[:, k, :])
                ht = hp.tile([_P, MF, n_pad], mm_dt, tag="ht")
                for m in range(MF):
                    pss = [
                        pp.tile([_P, tn], f32, tag="ps", name=f"ps_{m}_{i}")
                        for i, (t0, tn) in enumerate(tiles)
                    ]
                    for k in range(KH):
                        for i, (t0, tn) in enumerate(tiles):
                            nc.tensor.matmul(
                                pss[i][:],
                                w1s[:, k, m * _P:(m + 1) * _P],
                                xt[:, k, t0:t0 + tn],
                                start=(k == 0),
                                stop=(k == KH - 1),
                            )
                    for i, (t0, tn) in enumerate(tiles):
                        nc.scalar.activation(
                            ht[:, m, t0:t0 + tn], pss[i][:], AF.Relu, bias=b1s[:, m:m + 1]
                        )
                yt = yp.tile([_P, KH, n_pad], f32, tag="yt")
                for m2 in range(KH):
                    pss2 = [
                        pp.tile([_P, tn], f32, tag="ps", name=f"ps2_{m2}_{i}")
                        for i, (t0, tn) in enumerate(tiles)
                    ]
                    for k2 in range(MF):
                        for i, (t0, tn) in enumerate(tiles):
                            nc.tensor.matmul(
                                pss2[i][:],
                                w2s[:, k2, m2 * _P:(m2 + 1) * _P],
                                ht[:, k2, t0:t0 + tn],
                                start=(k2 == 0),
                                stop=(k2 == MF - 1),
                            )
                    for i, (t0, tn) in enumerate(tiles):
                        nc.scalar.activation(
                            yt[:, m2, t0:t0 + tn], pss2[i][:], AF.Identity, bias=b2s[:, m2:m2 + 1]
                        )
                for m2 in range(KH):
                    nc.sync.dma_start(yTr[:, m2, :], yt[:, m2, :])
                continue

            for t0, tn in tiles:
                sl = slice(t0, t0 + tn)
                nq = (tn + sub_n - 1) // sub_n
                xt = xp.tile([_P, KH, tn], mm_dt, tag="xt")
                if not skip_xdma:
                    nc.sync.dma_start(xt[:], xTr[:, :, sl])

                ht = hp.tile([_P, MF, tn], mm_dt, tag="ht")
                for m in range(MF):
                    ps = pp.tile([_P, tn], f32, tag="ps1")
                    for k in range(KH):
                        for q in range(nq):
                            qs = slice(q * sub_n, min((q + 1) * sub_n, tn))
                            nc.tensor.matmul(
                                ps[:, qs],
                                w1s[:, k, m * _P:(m + 1) * _P],
                                xt[:, k, qs],
                                start=(k == 0 and q == 0),
                                stop=(k == KH - 1 and q == nq - 1),
                                skip_group_check=True,
                            )
                    if not skip_act:
                        use_dve = evac == "dve" or (evac == "split" and m % 2 == 1)
                        if use_dve:
                            nc.vector.tensor_scalar(
                                ht[:, m, :], ps[:], b1s[:, m:m + 1], 0.0,
                                mybir.AluOpType.add, mybir.AluOpType.max,
                            )
                        else:
                            nc.scalar.activation(ht[:, m, :], ps[:], AF.Relu, bias=b1s[:, m:m + 1])

                yt = yp.tile([_P, KH, tn], f32, tag="yt")
                for m2 in range(KH):
                    ps2 = pp.tile([_P, tn], f32, tag="ps2")
                    for k2 in range(MF):
                        for q in range(nq):
                            qs = slice(q * sub_n, min((q + 1) * sub_n, tn))
                            nc.tensor.matmul(
                                ps2[:, qs],
                                w2s[:, k2, m2 * _P:(m2 + 1) * _P],
                                ht[:, k2, qs],
                                start=(k2 == 0 and q == 0),
                                stop=(k2 == MF - 1 and q == nq - 1),
                                skip_group_check=True,
                            )
                    if not skip_act:
                        use_dve = evac == "dve" or (evac == "split" and m2 % 2 == 1)
                        if use_dve:
                            nc.vector.tensor_scalar_add(yt[:, m2, :], ps2[:], b2s[:, m2:m2 + 1])
                        else:
                            nc.scalar.activation(yt[:, m2, :], ps2[:], AF.Identity, bias=b2s[:, m2:m2 + 1])
                if not skip_ydma:
                    nc.sync.dma_start(yTr[:, :, sl], yt[:])

    nc.compile()
    if style == "grouped":
        n_removed = _dedupe_ldweights(nc)
        import os as _os
        if _os.environ.get("DEBUG_LDW"):
            print(f"[kernel] deduped {n_removed} Ldweights")
    return nc


_MM_DTYPE = "float32r"
_MM_STYLE = "stream"


def _get_nc(n_pad, mm_dtype_name, repeats=1, loop=1, style="stream"):
    key = (n_pad, mm_dtype_name, repeats, loop, style)
    if key not in _NC_CACHE:
        _NC_CACHE[key] = _build(n_pad, mm_dtype_name, repeats, loop, style)
    return _NC_CACHE[key]


def _np_mm_dtype(mm_dtype_name):
    if mm_dtype_name == "bfloat16":
        import ml_dtypes

        return ml_dtypes.bfloat16
    return np.float32


def _prepare(x, b_seq, W1, B1, W2, B2, mm_dtype_name):
    """Host-side routing: returns (idx_per_core, n_pad, in_maps)."""
    np_dt = _np_mm_dtype(mm_dtype_name)
    x = np.asarray(x)
    flat_x = np.ascontiguousarray(x.reshape(-1, _H), dtype=np.float32)
    bs = np.asarray(b_seq).reshape(-1)

    # Route: behavior b -> cores 2b and 2b+1, tokens split evenly.
    idx_per_core = []
    for b in range(_NB):
        idx = np.nonzero(bs == b + 1)[0]
        h = (len(idx) + 1) // 2
        idx_per_core.append(idx[:h])
        idx_per_core.append(idx[h:])
    nmax = max(len(i) for i in idx_per_core)
    n_pad = ((max(256, nmax) + 15) // 16) * 16

    in_maps = []
    for c in range(_NCORES):
        beh = c // 2
        idx = idx_per_core[c]
        xT = np.zeros((_H, n_pad), np_dt)
        if len(idx):
            xT[:, :len(idx)] = flat_x[idx].T.astype(np_dt)
        in_maps.append({
            "xT": xT,
            "w1": np.ascontiguousarray(np.asarray(W1[beh]).astype(np_dt)),
            "w2": np.ascontiguousarray(np.asarray(W2[beh]).astype(np_dt)),
            "b1": np.ascontiguousarray(np.asarray(B1[beh], dtype=np.float32).reshape(_F // _P, _P).T),
            "b2": np.ascontiguousarray(np.asarray(B2[beh], dtype=np.float32).reshape(_H // _P, _P).T),
        })
    return idx_per_core, n_pad, in_maps


def kernel(x, b_seq, W1, B1, W2, B2, _repeats=1):
    global LAST_RESULTS
    import os

    from concourse.bass_utils import run_bass_kernel_spmd

    _dev = os.environ.get("BSPFF_DEV") == "1"
    mm_dtype = os.environ.get("MM_DTYPE", _MM_DTYPE) if _dev else _MM_DTYPE
    style = os.environ.get("MM_STYLE", _MM_STYLE) if _dev else _MM_STYLE
    idx_per_core, n_pad, in_maps = _prepare(x, b_seq, W1, B1, W2, B2, mm_dtype)
    nc = _get_nc(n_pad, mm_dtype, _repeats, style=style)

    res = run_bass_kernel_spmd(nc, in_maps, core_ids=list(range(_NCORES)))
    LAST_RESULTS = res

    out = np.zeros((_B * _T, _H), np.float32)
    for c in range(_NCORES):
        idx = idx_per_core[c]
        if len(idx):
            out[idx] = res.results[c]["yT"][:, :len(idx)].T
    return out.reshape(_B, _T, _H)
